# revision 3
# baseline (speedup 1.0000x reference)
"""AlignmentAttentionLayer Trainium2 kernel v3: v2 + software-pipelined
instruction emission so no engine queue ever stalls on a cross-engine latency.

Pipeline structure (emission order == per-engine queue order):
  - Y DMA issued 2 outer chunks ahead (ypool bufs=3).
  - h_n prep (PE transposes + whn matmul) for chunk c+1 during chunk c.
  - Inner loop: M-matmuls(t) emitted, then tail(t-1) = [tanh, s-matmul, exp,
    prods, reduces] - so the PE reaches s(t-1) only after M(t), by which
    time tanh(t-1) has long finished on ACT.
  - Out stage (z-replicate, reciprocal, scale, 18 matmuls, tanh, DMA) for
    chunk c-1 emitted near the end of chunk c's M-work.
"""

import numpy as np

B = 16384
D = 300
L = 50
NCORES = 8
BB = B // NCORES
P = 128
NB = 64
NCH = 8
NI = NB * L // NCH        # 400
RI = NI // L              # 8
ZP = 44                   # partition holding the z ones-row (es2 slice)

USE_FP8_S = False         # fp8 DoubleRow s-matmul


def _build(bb: int):
    import concourse.bass as bass
    import concourse.mybir as mybir
    from concourse.tile import TileContext
    from concourse.masks import make_identity
    from contextlib import ExitStack

    f32 = mybir.dt.float32
    bf16 = mybir.dt.bfloat16
    f8 = mybir.dt.float8e4
    AF = mybir.ActivationFunctionType
    OP = mybir.AluOpType
    AX = mybir.AxisListType

    SUBS = [(0, 128), (128, 128), (256, 44)]

    nc = bass.Bass("TRN2")
    Y_d = nc.declare_dram_parameter("Y", [bb, D, L], f32, isOutput=False)
    hn_d = nc.declare_dram_parameter("h_n", [bb, D], f32, isOutput=False)
    Wy_d = nc.declare_dram_parameter("W_y", [D, D], f32, isOutput=False)
    Wh_d = nc.declare_dram_parameter("W_h", [D, D], f32, isOutput=False)
    Wp_d = nc.declare_dram_parameter("W_p", [D, D], f32, isOutput=False)
    Wx_d = nc.declare_dram_parameter("W_x", [D, D], f32, isOutput=False)
    w_d = nc.declare_dram_parameter("w", [D], f32, isOutput=False)
    out_d = nc.declare_dram_parameter("out", [bb, D], f32, isOutput=True)

    chunks = bb // NB

    with TileContext(nc) as tc, ExitStack() as ctx:
        const = ctx.enter_context(tc.tile_pool(name="const", bufs=1))
        init = ctx.enter_context(tc.tile_pool(name="init", bufs=2))
        work = ctx.enter_context(tc.tile_pool(name="work", bufs=3))
        ypool = ctx.enter_context(tc.tile_pool(name="ypool", bufs=3))
        inner = ctx.enter_context(tc.tile_pool(name="inner", bufs=4))
        psM = ctx.enter_context(tc.tile_pool(name="psM", bufs=2, space="PSUM"))
        psS = ctx.enter_context(tc.tile_pool(name="psS", bufs=1, space="PSUM"))
        psA = ctx.enter_context(tc.tile_pool(name="psA", bufs=1, space="PSUM"))

        ident = const.tile([P, P], f32, tag="ident")
        make_identity(nc, ident)
        # selector = column ZP of identity: replicates rhs row ZP to all 128
        sel = const.tile([ZP + 1, P], f32, tag="sel")
        nc.vector.tensor_copy(out=sel[:, :],
                              in_=ident[:ZP + 1, ZP, None].to_broadcast((ZP + 1, P)))
        identB = const.tile([NB, NB], bf16, tag="identB")
        nc.vector.tensor_copy(out=identB[:], in_=ident[:NB, :NB])

        # ---- preload weights, transposed [e, d], bf16 ----
        wTs = {}
        for name, wd in (("wy", Wy_d), ("wh", Wh_d), ("wp", Wp_d), ("wx", Wx_d)):
            wT = const.tile([P, 3, D], bf16, tag=f"{name}T")
            tmp = init.tile([P, 3, D], f32, tag="wtmp")
            with nc.allow_non_contiguous_dma(reason="one-time 300x300 transpose load"):
                for es, (e0, pe) in enumerate(SUBS):
                    nc.scalar.dma_start(out=tmp[:pe, es, :],
                                        in_=wd[:, e0:e0 + pe].rearrange("d e -> e d"))
            for es, (e0, pe) in enumerate(SUBS):
                nc.gpsimd.tensor_copy(out=wT[:pe, es, :], in_=tmp[:pe, es, :])
            wTs[name] = wT
        wyT, whT, wpT, wxT = wTs["wy"], wTs["wh"], wTs["wp"], wTs["wx"]

        # ---- w replicated to 128 columns ----
        wv = const.tile([P, 3], f32, tag="wv")
        with nc.allow_non_contiguous_dma(reason="one-time 300-elem strided load"):
            for ds, (d0, pd) in enumerate(SUBS):
                nc.scalar.dma_start(out=wv[:pd, ds:ds + 1], in_=w_d[d0:d0 + pd, None])
        if USE_FP8_S:
            w_repl = const.tile([P, 2, P], f8, tag="w_repl")
            for kt in range(2):
                nc.vector.tensor_copy(out=w_repl[:, kt, :],
                                      in_=wv[:, kt, None].to_broadcast((P, P)))
            w_rep2 = const.tile([ZP, P], f8, tag="w_rep2")
            nc.vector.tensor_copy(out=w_rep2[:44, :],
                                  in_=wv[:44, 2, None].to_broadcast((44, P)))
        else:
            w_repl = const.tile([P, 3, P], bf16, tag="w_repl")
            for ds, (d0, pd) in enumerate(SUBS):
                nc.vector.tensor_copy(out=w_repl[:pd, ds, :],
                                      in_=wv[:pd, ds, None].to_broadcast((pd, P)))

        # ---- h_n for the whole core ----
        hn_all = const.tile([NB, chunks, D], f32, tag="hn_all")
        nc.sync.dma_start(out=hn_all[:], in_=hn_d.rearrange("(c p) e -> p c e", p=NB))

        # ---- pre-materialize Yb buffers: ones-row at partition ZP of es2 ----
        zb = (ZP // 32) * 32
        ybufs = []
        for _i in range(3):
            Yb0 = ypool.tile([P, 3, NB, L], bf16, tag="Yb")
            nc.vector.memset(Yb0[zb:zb + 32, 2], 1.0)
            nc.vector.tensor_copy(
                out=Yb0[64:128, 2],
                in_=ident[64:128, 64:128, None].to_broadcast((64, NB, L)))
            ybufs.append(Yb0)

        def emit_ydma(c):
            Yb = ypool.tile([P, 3, NB, L], bf16, tag="Yb")
            b0 = c * NB
            for es, (e0, pe) in enumerate(SUBS):
                nc.gpsimd.dma_start(
                    out=Yb[:pe, es],
                    in_=Y_d[b0:b0 + NB, e0:e0 + pe, :].rearrange("b e l -> e b l"))
            return Yb

        def emit_hnprep(c):
            hn = hn_all[:, c, :]
            hnT = work.tile([P, 3, NB], bf16, tag="hnT")
            for es, (e0, pe) in enumerate(SUBS):
                pt = psA.tile([P, 512], f32, tag="psa", name="pt")
                nc.tensor.transpose(pt[:pe, :NB], hn[:, e0:e0 + pe], ident[:NB, :NB])
                nc.vector.tensor_copy(out=hnT[:pe, es, :], in_=pt[:pe, :NB])
            whn_t = work.tile([NB, D], bf16, tag="whn_t")
            pwhn = psA.tile([P, 512], f32, tag="psa", name="pwhn")[:NB, :D]
            for es, (e0, pe) in enumerate(SUBS):
                nc.tensor.matmul(pwhn[:, :], hnT[:pe, es, :], whT[:pe, es, :],
                                 start=(es == 0), stop=(es == 2))
            nc.scalar.copy(out=whn_t[:, :], in_=pwhn[:, :])
            wy2x = work.tile([P, D], bf16, tag="wy2x")
            nc.vector.memset(wy2x[32:64, :], 0.0)
            nc.vector.tensor_copy(out=wy2x[:44, :], in_=wyT[:44, 2, :])
            nc.gpsimd.dma_start(out=wy2x[64:128, :], in_=whn_t[:, :])
            return hnT, whn_t, wy2x

        def emit_M(st, t):
            Yb, wy2x = st["Yb"], st["wy2x"]
            r0 = t * RI
            pm = psM.tile([P, 3, 512], f32, tag="pm")
            for ds, (d0, pd) in enumerate(SUBS):
                nc.tensor.matmul(pm[:pd, ds, :NI], wyT[:, 0, d0:d0 + pd],
                                 Yb[:, 0, r0:r0 + RI, :], start=True, stop=False)
                nc.tensor.matmul(pm[:pd, ds, :NI], wyT[:, 1, d0:d0 + pd],
                                 Yb[:, 1, r0:r0 + RI, :], start=False, stop=False)
                # es2 + bias fused: K=128 (wy es2 rows 0-43, zeros 44-63,
                # whn rows 64-127 against the identity-indicator block)
                nc.tensor.matmul(pm[:pd, ds, :NI], wy2x[:, d0:d0 + pd],
                                 Yb[:, 2, r0:r0 + RI, :], start=False, stop=True)
            return pm

        def emit_tail(st, t, pm):
            Yb, rT = st["Yb"], st["rT"]
            r0 = t * RI
            Mb = inner.tile([P, 3, NI], f8 if USE_FP8_S else bf16, tag="Mb")
            nc.scalar.activation(out=Mb[:, 0:2], in_=pm[:, 0:2, :NI], func=AF.Tanh)
            nc.scalar.activation(out=Mb[:44, 2], in_=pm[:44, 2, :NI], func=AF.Tanh)

            ps_s = psS.tile([P, 512], f32, tag="ps_s", name="ps_s")[:, :NI]
            if USE_FP8_S:
                nc.tensor.matmul(ps_s[:, :], w_repl[:, :, :], Mb[:, 0:2, :],
                                 start=True, stop=False,
                                 perf_mode=mybir.MatmulPerfMode.DoubleRow)
                nc.tensor.matmul(ps_s[:, :], w_rep2[:44, :], Mb[:44, 2, :],
                                 start=False, stop=True)
            else:
                for ds, (d0, pd) in enumerate(SUBS):
                    nc.tensor.matmul(ps_s[:, :], w_repl[:pd, ds, :], Mb[:pd, ds],
                                     start=(ds == 0), stop=(ds == 2))
            alpha = inner.tile([P, NI], bf16, tag="alpha")
            nc.scalar.activation(out=alpha[:], in_=ps_s[:], func=AF.Exp)

            al3 = alpha.rearrange("p (b l) -> p b l", l=L)
            prods = inner.tile([P, 3, RI, L], bf16, tag="prods")
            nc.vector.tensor_mul(out=prods[:, 0:2], in0=Yb[:, 0:2, r0:r0 + RI],
                                 in1=al3[:, None].to_broadcast((P, 2, RI, L)))
            nc.vector.tensor_mul(out=prods[:ZP + 1, 2],
                                 in0=Yb[:ZP + 1, 2, r0:r0 + RI], in1=al3[:ZP + 1])
            nc.vector.tensor_reduce(out=rT[:, 0:2, r0:r0 + RI], in_=prods[:, 0:2],
                                    axis=AX.X, op=OP.add)
            nc.vector.tensor_reduce(out=rT[:ZP + 1, 2, r0:r0 + RI],
                                    in_=prods[:ZP + 1, 2], axis=AX.X, op=OP.add)

        def emit_finish_a(st):
            rT = st["rT"]
            zinv = work.tile([P, NB], f32, tag="zinv")
            pz = psA.tile([P, 512], f32, tag="psa", name="pz")[:, :NB]
            nc.tensor.matmul(pz[:, :], sel[:ZP + 1, :], rT[:ZP + 1, 2, :],
                             start=True, stop=True)
            nc.vector.reciprocal(zinv[:], pz[:, :])
            rTb = work.tile([P, 3, NB], bf16, tag="rTb")
            for es, (e0, pe) in enumerate(SUBS):
                nc.vector.tensor_mul(out=rTb[:pe, es], in0=rT[:pe, es],
                                     in1=zinv[:pe, :])
            st["rTb"] = rTb

        def emit_finish(st):
            rT, hnT, rTb = st["rT"], st["hnT"], st["rTb"]
            b0 = st["c"] * NB
            ho = work.tile([NB, D], f32, tag="ho")
            for ds, (d0, pd) in enumerate(SUBS):
                ph = psA.tile([P, 512], f32, tag="psa", name="ph")[:NB, :pd]
                for es, (e0, pe) in enumerate(SUBS):
                    nc.tensor.matmul(ph[:, :], rTb[:pe, es, :],
                                     wpT[:pe, es, d0:d0 + pd],
                                     start=(es == 0), stop=False)

                for es, (e0, pe) in enumerate(SUBS):
                    nc.tensor.matmul(ph[:, :], hnT[:pe, es, :],
                                     wxT[:pe, es, d0:d0 + pd],
                                     start=False, stop=(es == 2))
                nc.scalar.activation(out=ho[:, d0:d0 + pd], in_=ph[:, :], func=AF.Tanh)
            nc.sync.dma_start(out=out_d[b0:b0 + NB, :], in_=ho[:])

        # ---- pipelined main loop ----
        ybq = [ybufs[0], ybufs[1]]  # Yb for chunks 0, 1 (DMA re-requested below)
        # re-issue DMAs into the pre-materialized buffers for chunks 0 and 1
        ybq = [emit_ydma(0), emit_ydma(1)]
        hnq = [emit_hnprep(0)]
        prev = None  # state of chunk c-1 awaiting finish
        for c in range(chunks):
            st = {"c": c, "Yb": ybq[0], "hnT": hnq[0][0], "whn_t": hnq[0][1], "wy2x": hnq[0][2]}
            ybq = ybq[1:]
            hnq = hnq[1:]
            if c + 2 < chunks + 2:
                if c + 2 < chunks:
                    ybq.append(emit_ydma(c + 2))
                else:
                    ybq.append(None)
            if c + 1 < chunks:
                hnq.append(emit_hnprep(c + 1))
            st["rT"] = work.tile([P, 3, NB], f32, tag="rT", name="rT")

            pm_prev = None
            for t in range(NCH):
                pm = emit_M(st, t)
                if t == NCH - 2 and prev is not None:
                    emit_finish_a(prev)
                if t == NCH - 1 and prev is not None:
                    emit_finish(prev)
                    prev = None
                if pm_prev is not None:
                    emit_tail(st, t - 1, pm_prev)
                pm_prev = pm
            if prev is not None:
                emit_finish_a(prev)
                emit_finish(prev)
                prev = None
            emit_tail(st, NCH - 1, pm_prev)
            prev = st
        emit_finish_a(prev)
        emit_finish(prev)

    return nc


_NC_CACHE = {}


def _install_walrus_workarounds():
    import json as _json
    import concourse.mybir as mybir
    import concourse.tile as ctile
    from concourse.tile import ScopedClock
    from concourse import bass_utils, bass2jax

    def _patched_drain_and_barrier(self, tick_clock, wait_clock):
        nc = self.nc
        collector = nc.sync.nop(nofuse=True)
        wait_clock.add_sem_waits(
            collector.ins, ScopedClock({None: tick_clock.global_clock}))
        si = collector.ins.sync_info
        waits = list(si.on_wait) if si is not None else []
        if len(waits) > 1:
            collector.ins.sync_info = mybir.SyncInfo(
                on_wait=[waits[0]], on_update=list(si.on_update))
            for w in waits[1:]:
                n = nc.sync.nop(nofuse=True)
                n.ins.sync_info = mybir.SyncInfo(on_wait=[w], on_update=[])
        nc.sync.drain()
        nc.all_engine_barrier()
        popped = nc._tile_sem_poison_stack.pop()
        assert popped is self._sem_poison
        nc.clear_and_free_semaphores(list(self.sems.allocated().values()))
        nc.all_engine_barrier()

    ctile.TileContext._drain_and_barrier = _patched_drain_and_barrier

    import os as _os
    if _os.environ.get("LDW_OPT", "0") == "1" and not getattr(
            bass_utils.run_command, "_ldw_wrapped", False):
        _orig_run = bass_utils.run_command

        def _run_command_ldw(cmd, *a, **kw):
            cmd = [c.replace("--enable-ldw-opt=false", "--enable-ldw-opt=true")
                   if isinstance(c, str) else c for c in cmd]
            return _orig_run(cmd, *a, **kw)

        _run_command_ldw._ldw_wrapped = True
        bass_utils.run_command = _run_command_ldw

    if getattr(bass_utils.compile_bir_kernel, "_wsplit_wrapped", False):
        return
    counter = [0]

    def _split_multiwait_bir(bir_json):
        bir = _json.loads(bir_json)
        changed = False
        for func in bir.get("functions", []):
            for blk in func.get("blocks", []):
                insts = blk.get("instructions")
                if not insts:
                    continue
                out = []
                for ins in insts:
                    si = ins.get("sync_info")
                    waits = (si or {}).get("on_wait") or []
                    if len(waits) > 1:
                        changed = True
                        for w in waits[:-1]:
                            counter[0] += 1
                            out.append({
                                "debug": ins.get("debug"),
                                "engine": ins["engine"],
                                "ins": [], "outs": [],
                                "name": f"I-wsplit-{counter[0]}",
                                "opcode": "NoOp",
                                "sync_info": {"on_update": [], "on_wait": [w]},
                            })
                        si["on_wait"] = [waits[-1]]
                    out.append(ins)
                blk["instructions"] = out
        return _json.dumps(bir).encode() if changed else bir_json

    _orig_compile = bass_utils.compile_bir_kernel

    def compile_bir_kernel(bir_json, tmpdir, neff_name="file.neff"):
        return _orig_compile(_split_multiwait_bir(bir_json), tmpdir, neff_name)

    compile_bir_kernel._wsplit_wrapped = True
    bass_utils.compile_bir_kernel = compile_bir_kernel
    bass2jax.compile_bir_kernel = compile_bir_kernel


def _get_nc(bb: int):
    if bb not in _NC_CACHE:
        _install_walrus_workarounds()
        _NC_CACHE[bb] = _build(bb)
    return _NC_CACHE[bb]


def kernel(Y, h_n, W_y, W_h, W_p, W_x, w, _collect=None):
    from concourse.bass_utils import run_bass_kernel_spmd

    Y = np.ascontiguousarray(np.asarray(Y, dtype=np.float32))
    h_n = np.ascontiguousarray(np.asarray(h_n, dtype=np.float32))
    W_y = np.ascontiguousarray(np.asarray(W_y, dtype=np.float32))
    W_h = np.ascontiguousarray(np.asarray(W_h, dtype=np.float32))
    W_p = np.ascontiguousarray(np.asarray(W_p, dtype=np.float32))
    W_x = np.ascontiguousarray(np.asarray(W_x, dtype=np.float32))
    w = np.ascontiguousarray(np.asarray(w, dtype=np.float32))

    bb = Y.shape[0] // NCORES
    nc = _get_nc(bb)
    in_maps = [
        {
            "Y": Y[i * bb:(i + 1) * bb],
            "h_n": h_n[i * bb:(i + 1) * bb],
            "W_y": W_y, "W_h": W_h, "W_p": W_p, "W_x": W_x, "w": w,
        }
        for i in range(NCORES)
    ]
    res = run_bass_kernel_spmd(nc, in_maps, core_ids=list(range(NCORES)))
    if _collect is not None:
        _collect.append(res)
    return np.concatenate([res.results[i]["out"] for i in range(NCORES)], axis=0)
